# revision 14
# baseline (speedup 1.0000x reference)
"""Trainium2 Bass kernel: CorrelatorK3 (V2).

Math (per batch b, one batch per NeuronCore):
    q0 = rbf_0 @ Q0_w.T + Q0_b          [N, N, F]
    q  = rbf_d @ Q_w.T  + Q_b
    r0 = rbf_0 @ R0_w.T + R0_b
    r  = rbf_d @ R_w.T  + R_b
    C[n, j] = sum_{i, f} (q0*q)[n, i, f] * (r0*r)[i, j, f] * 0.02

Layout: x = (row, col) flattened [N*N]; DMA assigns x = p*512 + c*32 +
q*8 + j2*2 + g (p partition, c chunk, q quarter).

Products R[f, y] (fp16, partitions 0-63 = Bm = r0*r, 64-127 = A = q0*q)
use y = rlo*256 + hi*128 + p where r = x & 511, rlo = r & 255,
hi = r >> 8. Consequences:
  - STT product writes are 128-contiguous runs (full DVE rate).
  - A-half (x=(n,i)): y = i*256 + (n&1)*128 + (n>>1) -- phase-2 moving
    for fixed i is one contiguous 256-run; consumed in i order, so it
    relocates to partitions 0-63 through a ring of full-rate
    SBUF->SBUF DMAs (contiguous both sides).
  - Bm-half (x=(i,j)): y = j*256 + (i&1)*128 + (i>>1) -- phase-2
    stationary [64, 128-j] at stride 256 (strided LDW ~205 ns, the one
    remaining penalty).

Phase 1 per (chunk, quarter): 8 f32r PE transposes -> pt psum; one Act
evac [128,1024] -> tt fp16; 4 fp16 proj matmuls as two row-group
concurrent pairs (g0 rows 0-63 / g1 rows 64-127, measured dstart ~5ns)
into pp0/ppd [128,1024]; bias s0 = pp0+b0 (Act/DVE alternating for
balance); one STT (ppd+bd)*s0 -> R (DVE, [128,1024]).

Phase 2: C^T[j, n'] = sum_i Bm^T[f,i,jh]^T @ A^T[f,:,i] with A-ring
moving (contiguous) and strided Bm stationary; psum accumulation over
all 256 i; then fp16 evac (x0.02), 4 PE transposes to C[n', j], fp32
evac, row-interleaved output DMA (n = 2*(n' & 127) + (n' >> 7)).
"""

import os
import sys

if "/opt/trn_rl_repo" not in sys.path:
    sys.path.insert(0, "/opt/trn_rl_repo")

from contextlib import ExitStack

_PHASES = os.environ.get("KERNEL_PHASES", "12")

import numpy as np

import concourse.mybir as mybir
import concourse.tile as tile
from concourse import bacc
from concourse.bass_utils import run_bass_kernel_spmd
from concourse.masks import make_identity

B, N, D, F = 8, 256, 64, 64
X = N * N
INTERVAL = 0.02

F32 = mybir.dt.float32
F32R = mybir.dt.float32r
F16 = mybir.dt.float16

CH = 16  # chunks per tensor (32 x-rows per partition each)
NQ = 4  # quarters per chunk


def _body(ctx, tc, rbf0, rbfd, w0, wd, b0, bd, cout):
    nc = tc.nc

    const = ctx.enter_context(tc.tile_pool(name="const", bufs=1))
    w0_sb = const.tile([128, 128], F16)
    wd_sb = const.tile([128, 128], F16)
    b0_sb = const.tile([128, 1], F32)
    bd_sb = const.tile([128, 1], F32)
    identf = const.tile([128, 128], F32)
    ident = const.tile([128, 128], F32R)
    identh = const.tile([128, 128], F16)

    res_pool = ctx.enter_context(tc.tile_pool(name="res", bufs=1))
    R = res_pool.tile([128, X], F16)

    if "1" in _PHASES:
        _phase1(
            tc, rbf0, rbfd, w0, wd, b0, bd,
            w0_sb, wd_sb, b0_sb, bd_sb, identf, ident, identh, R,
        )
    else:
        nc.gpsimd.memset(R[:], 0.0)
    if "2" in _PHASES:
        _phase2(tc, R, identh, cout)
    else:
        z = res_pool_out = None
        zt = tc.nc  # no-op; still need an output write
        co = ctx.enter_context(tc.tile_pool(name="co0", bufs=1))
        c0 = co.tile([128, 512], F32)
        nc.gpsimd.memset(c0[:], 0.0)
        nc.sync.dma_start(cout[0:128, :], c0[:, 0:256])
        nc.sync.dma_start(cout[128:256, :], c0[:, 256:512])


def _phase1(tc, rbf0, rbfd, w0, wd, b0, bd,
            w0_sb, wd_sb, b0_sb, bd_sb, identf, ident, identh, R):
    nc = tc.nc
    Copy = mybir.ActivationFunctionType.Copy
    Ident = mybir.ActivationFunctionType.Identity
    Alu = mybir.AluOpType

    # x = p*512 + c*32 + q*8 + j2*2 + g
    rbf0v = rbf0[:].bitcast(F32R).rearrange(
        "(p c j) d -> c p (j d)", p=128, c=CH
    )
    rbfdv = rbfd[:].bitcast(F32R).rearrange(
        "(p c j) d -> c p (j d)", p=128, c=CH
    )
    # product scatter view: y = rlo*256 + hi*128 + p with
    # rlo = jc8*8 + j2*2 + g, jc8 = (c&7)*4 + q, hi = c>=8.
    # Rv[:, jc8, hi, g] = [f, j2 @ 512, p @ 1]: 2 free dims, inner runs
    # of 128 contiguous elements.
    Rv = R[:].rearrange(
        "f (jc8 j2 g hi p) -> f jc8 hi g j2 p",
        jc8=32, j2=4, g=2, hi=2, p=128,
    )

    with (
        tc.tile_pool(name="chunk", bufs=3) as chunk_pool,
        tc.tile_pool(name="rbfT", bufs=3) as rbfT_pool,
        tc.tile_pool(name="s0p", bufs=4) as s0_pool,
        tc.tile_pool(name="pt", bufs=2, space="PSUM") as pt_pool,
        tc.tile_pool(name="pp0g0", bufs=1, space="PSUM") as pp0g0_pool,
        tc.tile_pool(name="pp0g1", bufs=1, space="PSUM") as pp0g1_pool,
        tc.tile_pool(name="ppdg0", bufs=1, space="PSUM") as ppdg0_pool,
        tc.tile_pool(name="ppdg1", bufs=1, space="PSUM") as ppdg1_pool,
    ):
        chunks = {}

        def fetch(c):
            ch0 = chunk_pool.tile([128, 2048], F32R, tag="ch0")
            chd = chunk_pool.tile([128, 2048], F32R, tag="chd")
            nc.sync.dma_start(ch0[:], rbf0v[c])
            nc.gpsimd.dma_start(chd[:], rbfdv[c])
            chunks[c] = (ch0, chd)

        # chunk 0 in quarter-granularity sub-DMAs; consts interleaved
        ch0 = chunk_pool.tile([128, 2048], F32R, tag="ch0")
        chd = chunk_pool.tile([128, 2048], F32R, tag="chd")
        chunks[0] = (ch0, chd)
        nc.sync.dma_start(ch0[:, 0:512], rbf0v[0][:, 0:512])
        nc.gpsimd.dma_start(chd[:, 0:512], rbfdv[0][:, 0:512])
        nc.sync.dma_start(w0_sb[:], w0[:])
        nc.sync.dma_start(wd_sb[:], wd[:])
        nc.sync.dma_start(b0_sb[:], b0[:])
        nc.sync.dma_start(bd_sb[:], bd[:])
        for sq in range(1, 4):
            sls = slice(512 * sq, 512 * (sq + 1))
            nc.sync.dma_start(ch0[:, sls], rbf0v[0][:, sls])
            nc.gpsimd.dma_start(chd[:, sls], rbfdv[0][:, sls])
        # f32r identity via round-copy (f32r memset fails the ISA check)
        make_identity(nc, identf[:])
        nc.scalar.activation(ident[:], identf[:], Copy)
        nc.scalar.activation(identh[:], identf[:], Copy)

        qi = 0
        for c in range(CH):
            if c + 1 < CH:
                fetch(c + 1)
            ch0, chd = chunks.pop(c)
            hi = 1 if c >= 8 else 0

            for q in range(NQ):
                jc8 = (c & 7) * 4 + q

                # 8 f32r transposes into one 2-bank psum tile
                pt = pt_pool.tile([128, 1024], F32, tag="pt")
                ptr = pt[:].bitcast(F32R)
                for bl in range(4):
                    sl = slice(128 * (4 * q + bl), 128 * (4 * q + bl + 1))
                    nc.tensor.transpose(
                        ptr[:, 128 * bl : 128 * (bl + 1)], ch0[:, sl],
                        ident[:],
                    )
                    nc.tensor.transpose(
                        ptr[:, 512 + 128 * bl : 512 + 128 * (bl + 1)],
                        chd[:, sl], ident[:],
                    )

                # evacuate to fp16: tt[0:64]=g0 rows, tt[64:128]=g1 rows;
                # cols 0:512 side-0, 512:1024 side-d
                tt = rbfT_pool.tile([128, 1024], F16, tag="tt")
                nc.scalar.activation(tt[:], pt[:], Copy)

                # projections: two row-group concurrent pairs with
                # per-group psum tiles so each group's bias/STT frees
                # its psum independently (short dependency cycles)
                pp0g = (
                    pp0g0_pool.tile([128, 512], F32, name="pp0g0", tag="pp0g0"),
                    pp0g1_pool.tile([128, 512], F32, name="pp0g1", tag="pp0g1"),
                )
                ppdg = (
                    ppdg0_pool.tile([128, 512], F32, name="ppdg0", tag="ppdg0"),
                    ppdg1_pool.tile([128, 512], F32, name="ppdg1", tag="ppdg1"),
                )
                nc.tensor.matmul(
                    pp0g[0][:], w0_sb[0:64, :], tt[0:64, 0:512],
                    start=True, stop=True, tile_position=(0, 0),
                )
                nc.tensor.matmul(
                    pp0g[1][:], w0_sb[64:128, :], tt[64:128, 0:512],
                    start=True, stop=True, tile_position=(64, 0),
                )
                nc.tensor.matmul(
                    ppdg[0][:], wd_sb[0:64, :], tt[0:64, 512:1024],
                    start=True, stop=True, tile_position=(0, 0),
                )
                nc.tensor.matmul(
                    ppdg[1][:], wd_sb[64:128, :], tt[64:128, 512:1024],
                    start=True, stop=True, tile_position=(64, 0),
                )

                # per-group bias + product; g0 bias on Act (queued right
                # after the evac), g1 bias alternating Act/DVE
                for g in range(2):
                    s0 = s0_pool.tile([128, 512], F32, tag=f"s0g{g}")
                    if g == 0 or qi % 2 == 0:
                        nc.scalar.activation(
                            s0[:], pp0g[g][:], Ident, bias=b0_sb[:]
                        )
                    else:
                        nc.vector.tensor_scalar_add(s0[:], pp0g[g][:], b0_sb[:])
                    nc.vector.scalar_tensor_tensor(
                        Rv[:, jc8, hi, g],
                        ppdg[g][:].rearrange("f (j2 p) -> f j2 p", j2=4),
                        bd_sb[:],
                        s0[:].rearrange("f (j2 p) -> f j2 p", j2=4),
                        Alu.add,
                        Alu.mult,
                    )
                qi += 1


def _phase2(tc, R, identh, cout):
    nc = tc.nc
    Copy = mybir.ActivationFunctionType.Copy
    IB = 32  # i per staged A block

    # Bm stationary view: [f, j, hi, p] (y = j*256 + hi*128 + p)
    Bs = R[0:64, :].rearrange("f (j hi p) -> f j hi p", j=256, hi=2, p=128)

    with (
        tc.tile_pool(name="aring", bufs=3) as ring_pool,
        tc.tile_pool(name="pc", bufs=1, space="PSUM") as pc_pool,
        tc.tile_pool(name="pt2", bufs=1, space="PSUM") as pt2_pool,
        tc.tile_pool(name="co", bufs=1) as co_pool,
    ):
        pc0 = pc_pool.tile([128, 256], F32, tag="pc0")  # j 0:128
        pc1 = pc_pool.tile([128, 256], F32, tag="pc1")  # j 128:256
        pcs = [pc0, pc1]
        rings = {}
        for k in range(N // IB):
            # stage A block k: R[64:128, 8192k:+8192] -> partitions 0-63
            ring = ring_pool.tile([64, IB * 256], F16, tag="ring")
            nc.sync.dma_start(
                ring[:], R[64:128, k * IB * 256 : (k + 1) * IB * 256]
            )
            rv = ring[:].rearrange("f (il n) -> f il n", il=IB)
            for il in range(IB):
                i = k * IB + il
                mv = rv[:, il, :]  # [64, 256] contiguous
                for jh in range(2):
                    nc.tensor.matmul(
                        pcs[jh][:],
                        Bs[:, 128 * jh : 128 * (jh + 1), i & 1, i >> 1],
                        mv,
                        start=(i == 0),
                        stop=(i == N - 1),
                        tile_position=(0, 0),
                    )

        # C^T[j, n'] -> C[n', j] via 4 fp16 transposes, then fp32 out.
        # n' = (n&1)*128 + (n>>1)  =>  n = 2*(n' & 127) + (n' >> 7)
        ct_sb = co_pool.tile([128, 512], F16)
        nc.scalar.activation(ct_sb[:, 0:256], pc0[:], Copy, scale=INTERVAL)
        nc.scalar.activation(ct_sb[:, 256:512], pc1[:], Copy, scale=INTERVAL)
        pt2 = pt2_pool.tile([128, 512], F16, tag="pt2")
        for b in range(2):  # n' half
            for jh in range(2):
                nc.tensor.transpose(
                    pt2[:, 256 * b + 128 * jh : 256 * b + 128 * jh + 128],
                    ct_sb[:, 256 * jh + 128 * b : 256 * jh + 128 * b + 128],
                    identh[:],
                )
        c2_sb = co_pool.tile([128, 512], F32)
        nc.scalar.activation(c2_sb[:], pt2[:], Copy)
        cview = cout[:].rearrange("(v n2) j -> n2 v j", v=128, n2=2)
        nc.sync.dma_start(cview[0], c2_sb[:, 0:256])
        nc.sync.dma_start(cview[1], c2_sb[:, 256:512])


def _build_nc():
    nc = bacc.Bacc("TRN2", target_bir_lowering=False)
    rbf0 = nc.dram_tensor("rbf0", [X, D], F32, kind="ExternalInput")
    rbfd = nc.dram_tensor("rbfd", [X, D], F32, kind="ExternalInput")
    w0 = nc.dram_tensor("w0", [128, 128], F16, kind="ExternalInput")
    wd = nc.dram_tensor("wd", [128, 128], F16, kind="ExternalInput")
    b0 = nc.dram_tensor("b0", [128, 1], F32, kind="ExternalInput")
    bd = nc.dram_tensor("bd", [128, 1], F32, kind="ExternalInput")
    cout = nc.dram_tensor("c", [N, N], F32, kind="ExternalOutput")
    with tile.TileContext(nc) as tc:
        with ExitStack() as ctx:
            _body(ctx, tc, rbf0, rbfd, w0, wd, b0, bd, cout)
    nc.compile()
    return nc


_CACHE = {}


def _get_nc():
    if "nc" not in _CACHE:
        _CACHE["nc"] = _build_nc()
    return _CACHE["nc"]


def _make_in_maps(inp):
    rbf_0 = np.ascontiguousarray(np.asarray(inp["rbf_0"], dtype=np.float32))
    rbf_d = np.ascontiguousarray(np.asarray(inp["rbf_d"], dtype=np.float32))

    # stacking: cols 0-63 = R-family (Bm products at psum parts 0-63),
    # 64-127 = Q-family (A products at 64-127); the [64, 128] block is
    # duplicated across partition halves for the two row-groups
    def wstack(wr, wq):
        wt = np.concatenate(
            [np.asarray(wr).T, np.asarray(wq).T], axis=1
        ).astype(np.float16)  # [64, 128]
        return np.concatenate([wt, wt], axis=0)  # [128, 128]

    def bstack(br, bq):
        return np.concatenate([np.asarray(br), np.asarray(bq)]).astype(
            np.float32
        )[:, None]  # [128, 1]

    w0 = wstack(inp["R0_w"], inp["Q0_w"])
    wd = wstack(inp["R_w"], inp["Q_w"])
    b0 = bstack(inp["R0_b"], inp["Q0_b"])
    bd = bstack(inp["R_b"], inp["Q_b"])

    return [
        {
            "rbf0": rbf_0[b].reshape(X, D),
            "rbfd": rbf_d[b].reshape(X, D),
            "w0": w0,
            "wd": wd,
            "b0": b0,
            "bd": bd,
        }
        for b in range(B)
    ]


def kernel(**inputs):
    in_maps = _make_in_maps(inputs)
    nc = _get_nc()
    res = run_bass_kernel_spmd(nc, in_maps, core_ids=list(range(B)))
    return np.stack([res.results[b]["c"] for b in range(B)], axis=0)


if __name__ == "__main__":
    import reference

    inp = {k: np.asarray(v) for k, v in reference.setup_inputs().items()}
    got = kernel(**inp)
    exp = np.asarray(reference.reference(**inp))
    err = np.abs(got - exp)
    print("absmax_err", err.max(), "rel", err.max() / np.abs(exp).max())
